# revision 3
# baseline (speedup 1.0000x reference)
"""Trainium2 Bass kernel for nn_MultiHeadAttention_50534585205084 (sparse pooled attention).

Sharding (8 cores): batch (4) x head-half (2). Core c handles batch c//2's
heads [8*(c%2), 8*(c%2)+8). Each core emits a PARTIAL final projection yT
[1024, 256] (pooled rows, transposed, bf16); the host sums the two halves per
batch, upsamples rows 8x (the reference's repeat+crop makes the final output
row-periodic with period KP=8), and adds bc.

On-chip algorithm (per core), all matmuls bf16 with fp32 PSUM accumulation:

  phase A (per tensor q/k/v): x is staged SEQ-MAJOR (xpad [2048, 1024] = 9
  zero rows + x[0:2039]; the last 9 x rows ride in a tiny 32-row boundary
  tensor xb). The causal depthwise conv (DK=3) + causal avg-pool (KP=8)
  commute with the dense projection, and decompose into 3 overlapping
  window-8 sums of RAW x (streams P_t[i] = sum_j xpad[8i+t+j], t=0..2):
      pooled[i,ch] = sum_t (w_t[ch]*sc/8) * (P_t[i] @ W)[ch] + bias[ch]
  The window sums are computed ON THE PE: for each seq-tile s (128 rows of
  xpad) and x-channel tile ct, one matmul with stationary lhsT = x-tile and a
  CONSTANT 0/1 pooling matrix [128, 48] as moving rhs produces all 48
  (window x stream) sums of that tile; windows straddling seq-tile
  boundaries get their residual rows from one extra matmul against xb, and
  the 16 spill columns are folded in during PSUM evacuation (ACT+DVE in
  parallel on per-half tiles; the tile scheduler coarsens waits to tile
  granularity, so every DMA chunk and evac half gets its own tile).
  The projection then contracts the evacuated streams [128, 768] against
  column-sharded W (de-expanded: tap scales are applied per OUTPUT channel
  from PSUM during the combine, so W ships once, not 3 tap-scaled copies),
  software-pipelined 2 cts behind the pooling with two oct passes in
  flight (PSUM: pooling 2x1 banks, projection 4x1, attention/output 2).

  phase B: per head: transposed logits E_T[m,n]=exp(qp.kp) with the causal
  mask accumulated on PE as an additive -30 strict-lower-triangular matrix;
  softmax denominator via a ones column in the vp lhsT; unnormalized
  out_T = vp_m @ E_T normalized with a partition-broadcast reciprocal;
  shared head up-projection Wup.

  phase C: yT = Wc_half.T-partial @ merged, ct-major accumulation.
"""
import sys
sys.path.insert(0, '/opt/trn_rl_repo')

from contextlib import ExitStack

import numpy as np
import ml_dtypes

import concourse.bass as bass
import concourse.mybir as mybir
import concourse.tile as tile
from concourse import bacc
from concourse.bass_utils import run_bass_kernel_spmd
from concourse.masks import make_identity

B, S, D, H, KP, DK = 4, 2048, 1024, 16, 8, 3
DD = D // H            # 64 head dim
N_CORES = 8
C = D // 2             # 512 channels per core (8 heads)
NP = S // KP           # 256 pooled positions
P = 128
NK = D // P            # 8 x-channel (contraction) tiles
NCT = C // P           # 4 output channel tiles (2 heads each)
NT = 16                # seq tiles of xpad in SBUF / window-triples per tile
NORM = float(DD) ** -0.25
XR = 2048              # xpad rows staged on host (9 zeros + x[0:2039]; the
                       # last 9 x rows only feed the spill tensor xb)

dt = mybir.dt
AF = mybir.ActivationFunctionType
OP = mybir.AluOpType


def _pool_mats():
    """Constant pooling matrices (bf16 0/1).

    pm [128, 48]: col 3j+t sums tile-local rows [8j+t, 8j+t+8) (clipped at
    128); pms [128, 2]: spill for the previous tile's triple j=15, t=1,2 —
    rows [0, t).
    """
    pm = np.zeros((P, 48), np.float32)
    for j in range(NT):
        for t in range(3):
            pm[8 * j + t: min(8 * j + t + 8, P), 3 * j + t] = 1.0
    # pms [32, 32]: spill matmul rhs. lhsT is the host-staged boundary tensor
    # xb [32, ch] with row p*16 + (s-1) = xpad[128*s + p] for s = 1..16,
    # p in {0,1}. Col 2*(s-1)+(t-1) for t in {1,2} is the residual of triple
    # (16*s - 1, t): the sum over this tile's rows p < t.
    pms = np.zeros((32, 32), np.float32)
    for si in range(16):
        for t in (1, 2):
            for p in range(t):
                pms[p * 16 + si, 2 * si + (t - 1)] = 1.0
    bf = ml_dtypes.bfloat16
    return pm.astype(bf), pms.astype(bf)


def _emit(nc, tc, aps):
    xs_ap = {nm: aps["x" + nm] for nm in "kqv"}
    w_ap = {nm: aps["w" + nm] for nm in "kqv"}
    wc, wup, mask, taps, bup2, pm, pms, yT = (
        aps["wc"], aps["wup"], aps["mask"], aps["taps"], aps["bup2"],
        aps["pm"], aps["pms"], aps["yT"])

    with ExitStack() as ctx:
        wpool = ctx.enter_context(tc.tile_pool(name="w", bufs=1))
        xpool = ctx.enter_context(tc.tile_pool(name="x", bufs=2))
        ptpool = ctx.enter_context(tc.tile_pool(name="pt", bufs=2))
        plpool = ctx.enter_context(tc.tile_pool(name="pl", bufs=1))
        apool = ctx.enter_context(tc.tile_pool(name="a", bufs=1))
        ypool = ctx.enter_context(tc.tile_pool(name="y", bufs=2))
        pp_ps = ctx.enter_context(tc.tile_pool(name="pp", bufs=2, space="PSUM"))
        pj_ps = ctx.enter_context(tc.tile_pool(name="pj", bufs=4, space="PSUM"))
        sm_ps = ctx.enter_context(tc.tile_pool(name="sm", bufs=2, space="PSUM"))

        w_sb = {}
        pooled = {}

        CW = 256               # x columns per chunk (512B rows: full DMA speed)

        def x_dma(nm, chunk):
            """One column-chunk of xpad -> two per-half tiles (the tile
            scheduler coarsens read-waits to tile granularity, so every DMA
            gets its own tile; halves let pooling start on the first 8 seq
            tiles while the rest is still in flight)."""
            src = xs_ap[nm][:, chunk * CW:(chunk + 1) * CW].rearrange(
                "(s p) c -> p s c", p=P)
            out = []
            for half in range(2):
                xc = xpool.tile([P, 8, CW], dt.bfloat16, tag=f"xc{chunk}{half}",
                                name=f"xc_{nm}{chunk}{half}")
                nc.sync.dma_start(xc[:], src[:, half * 8:half * 8 + 8, :])
                out.append(xc)
            return out

        # --- k's first x chunk owns the head of the HWDGE queue; small
        # constants go through the Pool SWDGE path (no HWDGE slot).
        xt_k = [x_dma("k", 0)]
        pm_sb = wpool.tile([P, 48], dt.bfloat16, tag="pm")
        nc.scalar.dma_start(pm_sb[:], pm[:])
        pms_sb = wpool.tile([32, 32], dt.bfloat16, tag="pms")
        nc.scalar.dma_start(pms_sb[:], pms[:])
        taps_sb = wpool.tile([P, NCT, 3, 4], dt.float32, tag="taps")
        nc.scalar.dma_start(taps_sb[:], taps.rearrange("p (o j s) -> p o j s", o=NCT, j=3))
        ident_sb = wpool.tile([P, P], dt.bfloat16, tag="ident")
        make_identity(nc, ident_sb[:])

        def pool_mm(nm, xt, xb, ct, half):
            """Window-sum matmuls for one half (8 seq tiles) of x-channel tile
            ct -> pt psum [128, 400] (384 triple cols + 16 spill cols)."""
            pt = pp_ps.tile([P, 400], dt.float32, tag="pp", name=f"pt_{nm}{ct}{half}")
            xc = xt[ct // 2][half]
            cols = slice((ct % 2) * P, (ct % 2) * P + P)
            for j in range(8):
                nc.tensor.matmul(pt[:, 48 * j:48 * j + 48], xc[:, j, cols],
                                 pm_sb[:], start=True, stop=True)
            nc.tensor.matmul(pt[:, 384:400], xb[:, ct * P:ct * P + P],
                             pms_sb[:, half * 16:half * 16 + 16],
                             start=True, stop=True)
            return pt

        def evac(nm, pts, pt_ps, ct, half):
            """PSUM -> SBUF bf16 + fold the 16 spill cols into their triples.

            lo/hi halves are separate per-ct tiles so the ACT and DVE copies
            run in parallel and proj(ct) waits only on ITS ct's evac.
            """
            t = ptpool.tile([P, 384], dt.bfloat16, tag=f"pt{half}{ct}",
                            name=f"pt{half}_{nm}{ct}")
            pts.setdefault(ct, [None, None])[half] = t
            if half == 0:
                nc.scalar.copy(t[:], pt_ps[:, 0:384])
            else:
                nc.vector.tensor_copy(t[:], pt_ps[:, 0:384])
            spill = pt_ps[:, 384:400].rearrange("p (s j) -> p s j", j=2)
            d = t[:].rearrange("p (s j) -> p s j", s=8)[:, :, 46:48]
            nc.vector.tensor_tensor(d, spill[:], d, op=OP.add)

        def phase_a(nm, xt=None, post_oct=None):
            """Pool + project + combine one tensor; pooled[nm] [128, NCT, 256]."""
            if xt is None:
                xt = [x_dma(nm, 0)]
            xb = xpool.tile([32, 1024], dt.bfloat16, tag="xb", name=f"xb_{nm}")
            nc.gpsimd.dma_start(xb[:], aps["xb" + nm][:])
            # weights in two halves so proj(ct<4) only waits the first one,
            # interleaved between x chunks (x has priority on the DMA pipe).
            wsb = [wpool.tile([P, NK // 2, C], dt.bfloat16, tag=f"w{nm}{i}",
                              name=f"w_{nm}{i}") for i in range(2)]
            w_sb[nm] = wsb
            wr = w_ap[nm].rearrange("(k p) c -> p k c", p=P)
            xt.append(x_dma(nm, 1))
            xt.append(x_dma(nm, 2))
            xt.append(x_dma(nm, 3))
            # weights after all x chunks: pooling is chunk-gated, projection
            # runs 2 cts late anyway, so x owns the head of the DMA pipe.
            nc.sync.dma_start(wsb[0][:], wr[:, 0:4, :])
            nc.sync.dma_start(wsb[1][:], wr[:, 4:8, :])

            pts = {}
            pl = plpool.tile([P, NCT, NP], dt.bfloat16, tag=f"pool_{nm}",
                             name=f"pool_{nm}")
            pooled[nm] = pl

            # one oct in flight: its lo/hi 1-bank psum pair rotates through
            # pj_ps while the next oct's matmuls start.
            def mk_pj(oct):
                return (pj_ps.tile([P, 384], dt.float32, tag="pj",
                                   name=f"pjl_{nm}{oct}"),
                        pj_ps.tile([P, 384], dt.float32, tag="pj",
                                   name=f"pjh_{nm}{oct}"))

            def proj(pjt, ct, oct):
                w = wsb[ct // 4][:, ct % 4, oct * P:oct * P + P]
                for part in range(2):
                    nc.tensor.matmul(pjt[part][:], w, pts[ct][part][:],
                                     start=(ct == 0), stop=(ct == NK - 1))

            # combine: pooled[:, oct, i] = sum_t s_t * pj[:, 3i+t] + bias
            pi = {"q": 0, "k": 1, "v": 2}[nm]
            HP = NP // 2

            def combine(pjt, oct):
                # stage 1: three PARALLEL psum reads (ACT/DVE/Pool) so the pj
                # psum pair frees after ~one op latency; stage 2: two cheap
                # bf16 adds off-psum on DVE.
                for half in range(2):
                    pj3 = pjt[half][:].rearrange("p (i t) -> p t i", t=3)
                    ns = slice(half * HP, half * HP + HP)
                    z1 = apool.tile([P, HP], dt.bfloat16, tag="cmb1",
                                    name=f"cmb1_{nm}{oct}{half}", bufs=4)
                    nc.scalar.activation(z1[:], pj3[:, 0, :], AF.Identity,
                                         bias=taps_sb[:, oct, pi, 3:4],
                                         scale=taps_sb[:, oct, pi, 0:1])
                    z2 = apool.tile([P, HP], dt.bfloat16, tag="cmb2",
                                    name=f"cmb2_{nm}{oct}{half}", bufs=4)
                    nc.vector.tensor_scalar(
                        z2[:], pj3[:, 1, :], taps_sb[:, oct, pi, 1:2], None,
                        op0=OP.mult)
                    z3 = apool.tile([P, HP], dt.bfloat16, tag="cmb3",
                                    name=f"cmb3_{nm}{oct}{half}", bufs=4)
                    nc.vector.tensor_scalar(
                        z3[:], pj3[:, 2, :], taps_sb[:, oct, pi, 2:3], None,
                        op0=OP.mult)
                    nc.gpsimd.tensor_tensor(z2[:], z1[:], z2[:], op=OP.add)
                    nc.gpsimd.tensor_tensor(pl[:, oct, ns], z2[:], z3[:], op=OP.add)

            # oct 0 software-pipelined one ct behind the pooling; octs 1-3
            # are pure PE passes over the evacuated streams.
            pj0, pj1 = mk_pj(0), mk_pj(1)
            for ct in range(NK):
                for half in range(2):
                    pt_ps = pool_mm(nm, xt, xb, ct, half)
                    evac(nm, pts, pt_ps, ct, half)
                if ct > 1:
                    proj(pj0, ct - 2, 0)
                    proj(pj1, ct - 2, 1)
            for ct in (NK - 2, NK - 1):
                proj(pj0, ct, 0)
                proj(pj1, ct, 1)
            combine(pj0, 0)
            combine(pj1, 1)
            # post_oct(i) is emitted one oct LATE so its PE ops (which wait on
            # combine(i)'s cross-engine chain) never stall the next oct pass.
            for oct in (2, 3):
                pjt = mk_pj(oct)
                for ct in range(NK):
                    proj(pjt, ct, oct)
                combine(pjt, oct)
                if post_oct is not None:
                    post_oct(oct - 2)
            if post_oct is not None:
                post_oct(2)
                post_oct(3)

        # --- phase A for k, q first (logits can start), then v.
        phase_a("k", xt=xt_k)
        mask_sb = wpool.tile([P, P], dt.bfloat16, tag="mask")
        nc.gpsimd.dma_start(mask_sb[:], mask[:])
        wup_sb = wpool.tile([DD, DD], dt.bfloat16, tag="wup")
        nc.gpsimd.dma_start(wup_sb[:], wup[:])
        bup2_sb = wpool.tile([P, 1], dt.float32, tag="bup2")
        nc.gpsimd.dma_start(bup2_sb[:], bup2[:])
        phase_a("q")

        # logits + exp for all 8 heads overlap v's phase A (sm_ps: 2 banks)
        hd = [dict() for _ in range(H // 2)]

        def logits(h):
            ct, half = h // 2, h % 2
            rows = slice(DD * half, DD * half + DD)
            hd[h]["ct"], hd[h]["rows"] = ct, rows
            qp_h = pooled["q"][rows, ct, :]
            kp_h = pooled["k"][rows, ct, :]
            psS0 = sm_ps.tile([P, NP], dt.float32, tag="sm", name=f"psS0_{h}")
            nc.tensor.matmul(psS0[:], kp_h[:, 0:P], qp_h[:, :], start=True, stop=False)
            nc.tensor.matmul(psS0[:, 0:P], ident_sb[:], mask_sb[:], start=False, stop=True)
            psS1 = sm_ps.tile([P, P], dt.float32, tag="sm", name=f"psS1_{h}")
            nc.tensor.matmul(psS1[:], kp_h[:, P:NP], qp_h[:, P:NP], start=True, stop=False)
            nc.tensor.matmul(psS1[:], ident_sb[:], mask_sb[:], start=False, stop=True)
            E0 = apool.tile([P, NP], dt.bfloat16, tag=f"E0_{h}", name=f"E0_{h}")
            nc.scalar.activation(E0[:], psS0[:], AF.Exp)
            E1 = apool.tile([P, P], dt.bfloat16, tag=f"E1_{h}", name=f"E1_{h}")
            nc.scalar.activation(E1[:], psS1[:], AF.Exp)
            hd[h]["E0"], hd[h]["E1"] = E0, E1

        for h in range(H // 2):
            logits(h)

        # --- phase B fused into v's per-oct passes: as soon as pooled_v's oct
        # (2 heads) is combined, transpose vp, run U/softmax/Wup for those
        # heads while v's later oct passes still occupy the PE.
        wc_sb = wpool.tile([P, NCT, D], dt.bfloat16, tag="wc", name="wc_sb")
        nc.sync.dma_start(wc_sb[:], wc.rearrange("(t p) d -> p t d", p=P))
        merged = [plpool.tile([P, NP], dt.bfloat16, tag=f"merged{ct}",
                              name=f"merged{ct}") for ct in range(NCT)]
        vph = [[apool.tile([P, DD + 1], dt.bfloat16, tag=f"vph{h}_{mb}",
                           name=f"vph{h}_{mb}") for mb in range(2)]
               for h in range(H // 2)]
        for h in range(H // 2):
            for mb in range(2):
                nc.gpsimd.memset(vph[h][mb][:, DD:DD + 1], 1.0)

        def attn_oct(ct):
            for mb in range(2):
                pst = sm_ps.tile([P, P], dt.bfloat16, tag="sm", name=f"pst{ct}{mb}")
                nc.tensor.transpose(
                    pst[:], pooled["v"][:, ct, mb * P:(mb + 1) * P], ident_sb[:])
                for half in range(2):
                    nc.scalar.copy(vph[2 * ct + half][mb][:, 0:DD],
                                   pst[:, DD * half:DD * half + DD])
            hh = (2 * ct, 2 * ct + 1)
            for h in hh:
                psU = sm_ps.tile([DD + 1, NP], dt.float32, tag="sm", name=f"psU_{h}")
                nc.tensor.matmul(psU[:], vph[h][0][:], hd[h]["E0"][:],
                                 start=True, stop=False)
                nc.tensor.matmul(psU[:, P:NP], vph[h][1][:], hd[h]["E1"][:],
                                 start=False, stop=True)
                hd[h]["psU"] = psU
            for h in hh:
                recip = apool.tile([1, NP], dt.float32, tag=f"recip_{h}",
                                   name=f"recip_{h}")
                nc.vector.reciprocal(recip[:], hd[h]["psU"][DD:DD + 1, :])
                rb = apool.tile([DD, NP], dt.float32, tag=f"rb_{h}", name=f"rb_{h}")
                nc.gpsimd.partition_broadcast(rb[:], recip[:])
                hd[h]["rb"] = rb
            for h in hh:
                outT = apool.tile([DD, NP], dt.bfloat16, tag=f"outT_{h}",
                                  name=f"outT_{h}")
                nc.vector.tensor_mul(outT[:], hd[h]["psU"][0:DD, :], hd[h]["rb"][:])
                hd[h]["outT"] = outT
            for h in hh:
                psP = sm_ps.tile([DD, NP], dt.float32, tag="sm", name=f"psP_{h}")
                nc.tensor.matmul(psP[:], wup_sb[:], hd[h]["outT"][:],
                                 start=True, stop=True)
                rows = hd[h]["rows"]
                nc.scalar.activation(merged[ct][rows, :], psP[:], AF.Identity,
                                     bias=bup2_sb[rows, :], scale=1.0)

        phase_a("v", post_oct=attn_oct)

        # --- phase C: yT = Wc_half partial @ merged, ct-major accumulation.
        ysb = [ypool.tile([P, 4, NP], dt.bfloat16, tag="y", name=f"y{i}")
               for i in range(2)]
        for grp in range(2):
            psY = {}
            for dti in range(grp * 4, grp * 4 + 4):
                psY[dti] = pj_ps.tile([P, NP], dt.float32, tag="pj",
                                      name=f"psY{dti}")
            for ct in range(NCT):
                for dti in range(grp * 4, grp * 4 + 4):
                    nc.tensor.matmul(
                        psY[dti][:], wc_sb[:, ct, dti * P:(dti + 1) * P],
                        merged[ct][:], start=(ct == 0), stop=(ct == NCT - 1))
            for dti in range(grp * 4, grp * 4 + 4):
                if dti % 2 == 0:
                    nc.scalar.copy(ysb[grp][:, dti % 4, :], psY[dti][:])
                else:
                    nc.vector.tensor_copy(ysb[grp][:, dti % 4, :], psY[dti][:])
                if dti % 2 == 1:
                    dst = yT[(dti - 1) * P:(dti + 1) * P, :].rearrange(
                        "(j p) n -> p j n", p=P)
                    eng = nc.sync if dti % 4 == 1 else nc.scalar
                    eng.dma_start(dst, ysb[grp][:, (dti % 4) - 1:(dti % 4) + 1, :])


def build():
    nc = bacc.Bacc("TRN2", target_bir_lowering=False, debug=False,
                   num_devices=N_CORES)
    aps = {}
    for nm in ("xq", "xk", "xv"):
        aps[nm] = nc.dram_tensor(nm, [XR, D], dt.bfloat16, kind="ExternalInput").ap()
    for nm in ("xbq", "xbk", "xbv"):
        aps[nm] = nc.dram_tensor(nm, [32, D], dt.bfloat16, kind="ExternalInput").ap()
    for nm in ("wq", "wk", "wv"):
        aps[nm] = nc.dram_tensor(nm, [D, C], dt.bfloat16, kind="ExternalInput").ap()
    aps["wc"] = nc.dram_tensor("wc", [C, D], dt.bfloat16, kind="ExternalInput").ap()
    aps["wup"] = nc.dram_tensor("wup", [DD, DD], dt.bfloat16, kind="ExternalInput").ap()
    aps["mask"] = nc.dram_tensor("mask", [P, P], dt.bfloat16, kind="ExternalInput").ap()
    aps["taps"] = nc.dram_tensor("taps", [P, NCT * 3 * 4], dt.float32,
                                 kind="ExternalInput").ap()
    aps["bup2"] = nc.dram_tensor("bup2", [P, 1], dt.float32, kind="ExternalInput").ap()
    aps["pm"] = nc.dram_tensor("pm", [P, 48], dt.bfloat16, kind="ExternalInput").ap()
    aps["pms"] = nc.dram_tensor("pms", [32, 32], dt.bfloat16, kind="ExternalInput").ap()
    aps["yT"] = nc.dram_tensor("yT", [D, NP], dt.bfloat16, kind="ExternalOutput").ap()
    with tile.TileContext(nc) as tc:
        _emit(nc, tc, aps)
    nc.compile()
    return nc


_BUILT = None


def _get_built():
    global _BUILT
    if _BUILT is None:
        _BUILT = build()
    return _BUILT


def make_in_maps(q, k, v, Wq, bq, Wk, bk, Wv, bv, Wup, bup, Wc, bc,
                 wcq, bcq, wck, bck, wcv, bcv):
    bf = ml_dtypes.bfloat16
    q, k, v = (np.asarray(x, np.float32) for x in (q, k, v))
    mask_np = (-30.0 * np.tril(np.ones((P, P), np.float32), -1)).astype(bf)
    pm_np, pms_np = _pool_mats()

    def xpad(x):  # [S, D] -> [XR, D] bf16 (9 zero rows + x[0:XR-9])
        out = np.zeros((XR, D), np.float32)
        out[9:XR] = x[0:XR - 9]
        return out.astype(bf)

    def xbound(x):  # [32, D]: row p*16+(s-1) = xpad[128*s + p] = x[128*s+p-9]
        out = np.zeros((32, D), np.float32)
        for p in range(2):
            for si in range(16):
                out[p * 16 + si] = x[128 * (si + 1) + p - 9]
        return out.astype(bf)

    in_maps = []
    for core in range(N_CORES):
        b, half = core // 2, core % 2
        cs = slice(half * C, half * C + C)
        taps = np.zeros((P, NCT, 3, 4), np.float32)
        for oct in range(NCT):
            ch = slice(half * C + oct * P, half * C + (oct + 1) * P)
            for pi, (cw, cb, db, sc) in enumerate((
                    (wcq, bcq, bq, NORM), (wck, bck, bk, NORM), (wcv, bcv, bv, 1.0))):
                w0, w1, w2 = np.asarray(cw, np.float32)[:, ch]
                taps[:, oct, pi, 0] = w0 * sc / KP
                taps[:, oct, pi, 1] = w1 * sc / KP
                taps[:, oct, pi, 2] = w2 * sc / KP
                taps[:, oct, pi, 3] = ((w0 + w1 + w2) * np.asarray(db, np.float32)[ch]
                                       * sc + np.asarray(cb, np.float32)[ch])
        in_maps.append({
            "xq": xpad(q[b]),
            "xk": xpad(k[b]),
            "xv": xpad(v[b]),
            "xbq": xbound(q[b]),
            "xbk": xbound(k[b]),
            "xbv": xbound(v[b]),
            "wq": np.asarray(Wq, np.float32)[:, cs].astype(bf),
            "wk": np.asarray(Wk, np.float32)[:, cs].astype(bf),
            "wv": np.asarray(Wv, np.float32)[:, cs].astype(bf),
            "wc": np.asarray(Wc, np.float32)[cs, :].astype(bf),
            "wup": np.asarray(Wup, np.float32).astype(bf),
            "mask": mask_np,
            "taps": taps.reshape(P, NCT * 3 * 4),
            "bup2": np.tile(np.asarray(bup, np.float32), 2).reshape(P, 1),
            "pm": pm_np,
            "pms": pms_np,
        })
    return in_maps


def gather(results, bc):
    out = np.empty((B, S, D), np.float32)
    bc = np.asarray(bc, np.float32)
    for b in range(B):
        y = (np.asarray(results[2 * b]["yT"], np.float32)
             + np.asarray(results[2 * b + 1]["yT"], np.float32))  # [D, NP]
        out[b] = np.repeat(y.T, KP, axis=0) + bc[None, :]
    return out


def kernel(q, k, v, Wq, bq, Wk, bk, Wv, bv, Wup, bup, Wc, bc,
           wcq, bcq, wck, bck, wcv, bcv):
    nc = _get_built()
    in_maps = make_in_maps(q, k, v, Wq, bq, Wk, bk, Wv, bv, Wup, bup, Wc, bc,
                           wcq, bcq, wck, bck, wcv, bcv)
    res = run_bass_kernel_spmd(nc, in_maps, core_ids=list(range(N_CORES)),
                               trace=False)
    return gather(res.results, bc)


# revision 4
# speedup vs baseline: 1.0269x; 1.0269x over previous
"""Trainium2 Bass kernel for nn_MultiHeadAttention_50534585205084 (sparse pooled attention).

Sharding (8 cores): batch (4) x head-half (2). Core c handles batch c//2's
heads [8*(c%2), 8*(c%2)+8). Each core emits a PARTIAL final projection yT
[1024, 256] (pooled rows, transposed, bf16); the host sums the two halves per
batch, upsamples rows 8x (the reference's repeat+crop makes the final output
row-periodic with period KP=8), and adds bc.

On-chip algorithm (per core), all matmuls bf16 with fp32 PSUM accumulation:

  phase A (per tensor q/k/v): x is staged SEQ-MAJOR (xpad [2048, 1024] = 9
  zero rows + x[0:2039]; the last 9 x rows ride in a tiny 32-row boundary
  tensor xb). The causal depthwise conv (DK=3) + causal avg-pool (KP=8)
  commute with the dense projection, and decompose into 3 overlapping
  window-8 sums of RAW x (streams P_t[i] = sum_j xpad[8i+t+j], t=0..2):
      pooled[i,ch] = sum_t (w_t[ch]*sc/8) * (P_t[i] @ W)[ch] + bias[ch]
  The window sums are computed ON THE PE: for each seq-tile s (128 rows of
  xpad) and x-channel tile ct, one matmul with stationary lhsT = x-tile and a
  CONSTANT 0/1 pooling matrix [128, 48] as moving rhs produces all 48
  (window x stream) sums of that tile; windows straddling seq-tile
  boundaries get their residual rows from one extra matmul against xb, and
  the 16 spill columns are folded in during PSUM evacuation (ACT+DVE in
  parallel on per-half tiles; the tile scheduler coarsens waits to tile
  granularity, so every DMA chunk and evac half gets its own tile).
  The projection then contracts the evacuated streams [128, 768] against
  column-sharded W (de-expanded: tap scales are applied per OUTPUT channel
  from PSUM during the combine, so W ships once, not 3 tap-scaled copies),
  software-pipelined 2 cts behind the pooling with two oct passes in
  flight (PSUM: pooling 2x1 banks, projection 4x1, attention/output 2).

  phase B: per head: transposed logits E_T[m,n]=exp(qp.kp) with the causal
  mask accumulated on PE as an additive -30 strict-lower-triangular matrix;
  softmax denominator via a ones column in the vp lhsT; unnormalized
  out_T = vp_m @ E_T normalized with a partition-broadcast reciprocal;
  shared head up-projection Wup.

  phase C: yT = Wc_half.T-partial @ merged, ct-major accumulation.
"""
import sys
sys.path.insert(0, '/opt/trn_rl_repo')

from contextlib import ExitStack

import numpy as np
import ml_dtypes

import concourse.bass as bass
import concourse.mybir as mybir
import concourse.tile as tile
from concourse import bacc
from concourse.bass_utils import run_bass_kernel_spmd
from concourse.masks import make_identity

B, S, D, H, KP, DK = 4, 2048, 1024, 16, 8, 3
DD = D // H            # 64 head dim
N_CORES = 8
C = D // 2             # 512 channels per core (8 heads)
NP = S // KP           # 256 pooled positions
P = 128
NK = D // P            # 8 x-channel (contraction) tiles
NCT = C // P           # 4 output channel tiles (2 heads each)
NT = 16                # seq tiles of xpad in SBUF / window-triples per tile
NORM = float(DD) ** -0.25
XR = 2048              # xpad rows staged on host (9 zeros + x[0:2039]; the
                       # last 9 x rows only feed the spill tensor xb)

dt = mybir.dt
AF = mybir.ActivationFunctionType
OP = mybir.AluOpType


def _pool_mats():
    """Constant pooling matrices (bf16 0/1).

    pm [128, 48]: col 3j+t sums tile-local rows [8j+t, 8j+t+8) (clipped at
    128); pms [128, 2]: spill for the previous tile's triple j=15, t=1,2 —
    rows [0, t).
    """
    pm = np.zeros((P, 48), np.float32)
    for j in range(NT):
        for t in range(3):
            pm[8 * j + t: min(8 * j + t + 8, P), 3 * j + t] = 1.0
    # pms [32, 32]: spill matmul rhs. lhsT is the host-staged boundary tensor
    # xb [32, ch] with row p*16 + (s-1) = xpad[128*s + p] for s = 1..16,
    # p in {0,1}. Col 2*(s-1)+(t-1) for t in {1,2} is the residual of triple
    # (16*s - 1, t): the sum over this tile's rows p < t.
    pms = np.zeros((32, 32), np.float32)
    for si in range(16):
        for t in (1, 2):
            for p in range(t):
                pms[p * 16 + si, 2 * si + (t - 1)] = 1.0
    bf = ml_dtypes.bfloat16
    return pm.astype(bf), pms.astype(bf)


def _emit(nc, tc, aps):
    xs_ap = {nm: aps["x" + nm] for nm in "kqv"}
    w_ap = {nm: aps["w" + nm] for nm in "kqv"}
    wc, wup, mask, taps, bup2, pm, pms, yT = (
        aps["wc"], aps["wup"], aps["mask"], aps["taps"], aps["bup2"],
        aps["pm"], aps["pms"], aps["yT"])

    with ExitStack() as ctx:
        wpool = ctx.enter_context(tc.tile_pool(name="w", bufs=1))
        xpool = ctx.enter_context(tc.tile_pool(name="x", bufs=2))
        ptpool = ctx.enter_context(tc.tile_pool(name="pt", bufs=2))
        plpool = ctx.enter_context(tc.tile_pool(name="pl", bufs=1))
        apool = ctx.enter_context(tc.tile_pool(name="a", bufs=1))
        ypool = ctx.enter_context(tc.tile_pool(name="y", bufs=2))
        pp_ps = ctx.enter_context(tc.tile_pool(name="pp", bufs=2, space="PSUM"))
        pj_ps = ctx.enter_context(tc.tile_pool(name="pj", bufs=4, space="PSUM"))
        sm_ps = ctx.enter_context(tc.tile_pool(name="sm", bufs=2, space="PSUM"))

        w_sb = {}
        pooled = {}

        CW = 256               # x columns per chunk (512B rows: full DMA speed)

        def x_dma(nm, chunk):
            """One column-chunk of xpad -> two per-half tiles (the tile
            scheduler coarsens read-waits to tile granularity, so every DMA
            gets its own tile; halves let pooling start on the first 8 seq
            tiles while the rest is still in flight)."""
            src = xs_ap[nm][:, chunk * CW:(chunk + 1) * CW].rearrange(
                "(s p) c -> p s c", p=P)
            out = []
            for half in range(2):
                xc = xpool.tile([P, 8, CW], dt.bfloat16, tag=f"xc{chunk}{half}",
                                name=f"xc_{nm}{chunk}{half}")
                nc.sync.dma_start(xc[:], src[:, half * 8:half * 8 + 8, :])
                out.append(xc)
            return out

        # --- k's first x chunk owns the head of the HWDGE queue; small
        # constants go through the Pool SWDGE path (no HWDGE slot).
        xt_k = [x_dma("k", 0)]
        pm_sb = wpool.tile([P, 48], dt.bfloat16, tag="pm")
        nc.scalar.dma_start(pm_sb[:], pm[:])
        pms_sb = wpool.tile([32, 32], dt.bfloat16, tag="pms")
        nc.scalar.dma_start(pms_sb[:], pms[:])
        taps_sb = wpool.tile([P, NCT, 3, 4], dt.float32, tag="taps")
        nc.scalar.dma_start(taps_sb[:], taps.rearrange("p (o j s) -> p o j s", o=NCT, j=3))
        ident_sb = wpool.tile([P, P], dt.bfloat16, tag="ident")
        make_identity(nc, ident_sb[:])

        def pool_mm(nm, xt, xb, ct, half):
            """Window-sum matmuls for one half (8 seq tiles) of x-channel tile
            ct -> pt psum [128, 400] (384 triple cols + 16 spill cols)."""
            pt = pp_ps.tile([P, 400], dt.float32, tag="pp", name=f"pt_{nm}{ct}{half}")
            xc = xt[ct // 2][half]
            cols = slice((ct % 2) * P, (ct % 2) * P + P)
            for j in range(8):
                nc.tensor.matmul(pt[:, 48 * j:48 * j + 48], xc[:, j, cols],
                                 pm_sb[:], start=True, stop=True)
            nc.tensor.matmul(pt[:, 384:400], xb[:, ct * P:ct * P + P],
                             pms_sb[:, half * 16:half * 16 + 16],
                             start=True, stop=True)
            return pt

        def evac(nm, pts, pt_ps, ct, half):
            """PSUM -> SBUF bf16 + fold the 16 spill cols into their triples.

            lo/hi halves are separate per-ct tiles so the ACT and DVE copies
            run in parallel and proj(ct) waits only on ITS ct's evac.
            """
            t = ptpool.tile([P, 384], dt.bfloat16, tag=f"pt{half}{ct}",
                            name=f"pt{half}_{nm}{ct}")
            pts.setdefault(ct, [None, None])[half] = t
            if half == 0:
                nc.scalar.copy(t[:], pt_ps[:, 0:384])
            else:
                nc.vector.tensor_copy(t[:], pt_ps[:, 0:384])
            spill = pt_ps[:, 384:400].rearrange("p (s j) -> p s j", j=2)
            d = t[:].rearrange("p (s j) -> p s j", s=8)[:, :, 46:48]
            nc.vector.tensor_tensor(d, spill[:], d, op=OP.add)

        def phase_a(nm, xt=None, post_oct=None):
            """Pool + project + combine one tensor; pooled[nm] [128, NCT, 256]."""
            if xt is None:
                xt = [x_dma(nm, 0)]
            xb = xpool.tile([32, 1024], dt.bfloat16, tag="xb", name=f"xb_{nm}")
            nc.gpsimd.dma_start(xb[:], aps["xb" + nm][:])
            # weights in two halves so proj(ct<4) only waits the first one,
            # interleaved between x chunks (x has priority on the DMA pipe).
            wsb = [wpool.tile([P, NK // 2, C], dt.bfloat16, tag=f"w{nm}{i}",
                              name=f"w_{nm}{i}") for i in range(2)]
            w_sb[nm] = wsb
            wr = w_ap[nm].rearrange("(k p) c -> p k c", p=P)
            xt.append(x_dma(nm, 1))
            nc.sync.dma_start(wsb[0][:], wr[:, 0:4, :])
            xt.append(x_dma(nm, 2))
            xt.append(x_dma(nm, 3))
            nc.sync.dma_start(wsb[1][:], wr[:, 4:8, :])

            pts = {}
            pl = plpool.tile([P, NCT, NP], dt.bfloat16, tag=f"pool_{nm}",
                             name=f"pool_{nm}")
            pooled[nm] = pl

            # one oct in flight: its lo/hi 1-bank psum pair rotates through
            # pj_ps while the next oct's matmuls start.
            def mk_pj(oct):
                return (pj_ps.tile([P, 384], dt.float32, tag="pj",
                                   name=f"pjl_{nm}{oct}"),
                        pj_ps.tile([P, 384], dt.float32, tag="pj",
                                   name=f"pjh_{nm}{oct}"))

            def proj(pjt, ct, oct):
                w = wsb[ct // 4][:, ct % 4, oct * P:oct * P + P]
                for part in range(2):
                    nc.tensor.matmul(pjt[part][:], w, pts[ct][part][:],
                                     start=(ct == 0), stop=(ct == NK - 1))

            # combine: pooled[:, oct, i] = sum_t s_t * pj[:, 3i+t] + bias
            pi = {"q": 0, "k": 1, "v": 2}[nm]
            HP = NP // 2

            def combine(pjt, oct):
                # stage 1: three PARALLEL psum reads (ACT/DVE/Pool) so the pj
                # psum pair frees after ~one op latency; stage 2: two cheap
                # bf16 adds off-psum on DVE.
                for half in range(2):
                    pj3 = pjt[half][:].rearrange("p (i t) -> p t i", t=3)
                    ns = slice(half * HP, half * HP + HP)
                    z1 = apool.tile([P, HP], dt.bfloat16, tag="cmb1",
                                    name=f"cmb1_{nm}{oct}{half}", bufs=4)
                    nc.scalar.activation(z1[:], pj3[:, 0, :], AF.Identity,
                                         bias=taps_sb[:, oct, pi, 3:4],
                                         scale=taps_sb[:, oct, pi, 0:1])
                    z2 = apool.tile([P, HP], dt.bfloat16, tag="cmb2",
                                    name=f"cmb2_{nm}{oct}{half}", bufs=4)
                    nc.vector.tensor_scalar(
                        z2[:], pj3[:, 1, :], taps_sb[:, oct, pi, 1:2], None,
                        op0=OP.mult)
                    z3 = apool.tile([P, HP], dt.bfloat16, tag="cmb3",
                                    name=f"cmb3_{nm}{oct}{half}", bufs=4)
                    nc.vector.tensor_scalar(
                        z3[:], pj3[:, 2, :], taps_sb[:, oct, pi, 2:3], None,
                        op0=OP.mult)
                    nc.gpsimd.tensor_tensor(z2[:], z1[:], z2[:], op=OP.add)
                    nc.gpsimd.tensor_tensor(pl[:, oct, ns], z2[:], z3[:], op=OP.add)

            # oct 0 software-pipelined one ct behind the pooling; octs 1-3
            # are pure PE passes over the evacuated streams.
            pj0, pj1 = mk_pj(0), mk_pj(1)
            for ct in range(NK):
                for half in range(2):
                    pt_ps = pool_mm(nm, xt, xb, ct, half)
                    evac(nm, pts, pt_ps, ct, half)
                if ct > 1:
                    proj(pj0, ct - 2, 0)
                    proj(pj1, ct - 2, 1)
            for ct in (NK - 2, NK - 1):
                proj(pj0, ct, 0)
                proj(pj1, ct, 1)
            combine(pj0, 0)
            combine(pj1, 1)
            # post_oct(i) is emitted one oct LATE so its PE ops (which wait on
            # combine(i)'s cross-engine chain) never stall the next oct pass.
            for oct in (2, 3):
                pjt = mk_pj(oct)
                for ct in range(NK):
                    proj(pjt, ct, oct)
                combine(pjt, oct)
                if post_oct is not None:
                    post_oct(oct - 2)
            if post_oct is not None:
                post_oct(2)
                post_oct(3)

        # --- phase A for k, q first (logits can start), then v.
        phase_a("k", xt=xt_k)
        mask_sb = wpool.tile([P, P], dt.bfloat16, tag="mask")
        nc.gpsimd.dma_start(mask_sb[:], mask[:])
        wup_sb = wpool.tile([DD, DD], dt.bfloat16, tag="wup")
        nc.gpsimd.dma_start(wup_sb[:], wup[:])
        bup2_sb = wpool.tile([P, 1], dt.float32, tag="bup2")
        nc.gpsimd.dma_start(bup2_sb[:], bup2[:])
        phase_a("q")

        # logits + exp for all 8 heads overlap v's phase A (sm_ps: 2 banks)
        hd = [dict() for _ in range(H // 2)]

        def logits(h):
            ct, half = h // 2, h % 2
            rows = slice(DD * half, DD * half + DD)
            hd[h]["ct"], hd[h]["rows"] = ct, rows
            qp_h = pooled["q"][rows, ct, :]
            kp_h = pooled["k"][rows, ct, :]
            psS0 = sm_ps.tile([P, NP], dt.float32, tag="sm", name=f"psS0_{h}")
            nc.tensor.matmul(psS0[:], kp_h[:, 0:P], qp_h[:, :], start=True, stop=False)
            nc.tensor.matmul(psS0[:, 0:P], ident_sb[:], mask_sb[:], start=False, stop=True)
            psS1 = sm_ps.tile([P, P], dt.float32, tag="sm", name=f"psS1_{h}")
            nc.tensor.matmul(psS1[:], kp_h[:, P:NP], qp_h[:, P:NP], start=True, stop=False)
            nc.tensor.matmul(psS1[:], ident_sb[:], mask_sb[:], start=False, stop=True)
            E0 = apool.tile([P, NP], dt.bfloat16, tag=f"E0_{h}", name=f"E0_{h}")
            nc.scalar.activation(E0[:], psS0[:], AF.Exp)
            E1 = apool.tile([P, P], dt.bfloat16, tag=f"E1_{h}", name=f"E1_{h}")
            nc.scalar.activation(E1[:], psS1[:], AF.Exp)
            hd[h]["E0"], hd[h]["E1"] = E0, E1

        for h in range(H // 2):
            logits(h)

        # --- phase B fused into v's per-oct passes: as soon as pooled_v's oct
        # (2 heads) is combined, transpose vp, run U/softmax/Wup for those
        # heads while v's later oct passes still occupy the PE.
        wc_sb = wpool.tile([P, NCT, D], dt.bfloat16, tag="wc", name="wc_sb")
        nc.sync.dma_start(wc_sb[:], wc.rearrange("(t p) d -> p t d", p=P))
        merged = [plpool.tile([P, NP], dt.bfloat16, tag=f"merged{ct}",
                              name=f"merged{ct}") for ct in range(NCT)]
        vph = [[apool.tile([P, DD + 1], dt.bfloat16, tag=f"vph{h}_{mb}",
                           name=f"vph{h}_{mb}") for mb in range(2)]
               for h in range(H // 2)]
        for h in range(H // 2):
            for mb in range(2):
                nc.gpsimd.memset(vph[h][mb][:, DD:DD + 1], 1.0)

        def attn_oct(ct):
            for mb in range(2):
                pst = sm_ps.tile([P, P], dt.bfloat16, tag="sm", name=f"pst{ct}{mb}")
                nc.tensor.transpose(
                    pst[:], pooled["v"][:, ct, mb * P:(mb + 1) * P], ident_sb[:])
                for half in range(2):
                    nc.scalar.copy(vph[2 * ct + half][mb][:, 0:DD],
                                   pst[:, DD * half:DD * half + DD])
            hh = (2 * ct, 2 * ct + 1)
            for h in hh:
                psU = sm_ps.tile([DD + 1, NP], dt.float32, tag="sm", name=f"psU_{h}")
                nc.tensor.matmul(psU[:], vph[h][0][:], hd[h]["E0"][:],
                                 start=True, stop=False)
                nc.tensor.matmul(psU[:, P:NP], vph[h][1][:], hd[h]["E1"][:],
                                 start=False, stop=True)
                hd[h]["psU"] = psU
            for h in hh:
                recip = apool.tile([1, NP], dt.float32, tag=f"recip_{h}",
                                   name=f"recip_{h}")
                nc.vector.reciprocal(recip[:], hd[h]["psU"][DD:DD + 1, :])
                rb = apool.tile([DD, NP], dt.float32, tag=f"rb_{h}", name=f"rb_{h}")
                nc.gpsimd.partition_broadcast(rb[:], recip[:])
                hd[h]["rb"] = rb
            for h in hh:
                outT = apool.tile([DD, NP], dt.bfloat16, tag=f"outT_{h}",
                                  name=f"outT_{h}")
                nc.vector.tensor_mul(outT[:], hd[h]["psU"][0:DD, :], hd[h]["rb"][:])
                hd[h]["outT"] = outT
            for h in hh:
                psP = sm_ps.tile([DD, NP], dt.float32, tag="sm", name=f"psP_{h}")
                nc.tensor.matmul(psP[:], wup_sb[:], hd[h]["outT"][:],
                                 start=True, stop=True)
                rows = hd[h]["rows"]
                nc.scalar.activation(merged[ct][rows, :], psP[:], AF.Identity,
                                     bias=bup2_sb[rows, :], scale=1.0)

        phase_a("v", post_oct=attn_oct)

        # --- phase C: yT = Wc_half partial @ merged, ct-major accumulation.
        ysb = [ypool.tile([P, 4, NP], dt.bfloat16, tag="y", name=f"y{i}")
               for i in range(2)]
        for grp in range(2):
            psY = {}
            for dti in range(grp * 4, grp * 4 + 4):
                psY[dti] = pj_ps.tile([P, NP], dt.float32, tag="pj",
                                      name=f"psY{dti}")
            for ct in range(NCT):
                for dti in range(grp * 4, grp * 4 + 4):
                    nc.tensor.matmul(
                        psY[dti][:], wc_sb[:, ct, dti * P:(dti + 1) * P],
                        merged[ct][:], start=(ct == 0), stop=(ct == NCT - 1))
            for dti in range(grp * 4, grp * 4 + 4):
                if dti % 2 == 0:
                    nc.scalar.copy(ysb[grp][:, dti % 4, :], psY[dti][:])
                else:
                    nc.vector.tensor_copy(ysb[grp][:, dti % 4, :], psY[dti][:])
                if dti % 2 == 1:
                    dst = yT[(dti - 1) * P:(dti + 1) * P, :].rearrange(
                        "(j p) n -> p j n", p=P)
                    eng = nc.sync if dti % 4 == 1 else nc.scalar
                    eng.dma_start(dst, ysb[grp][:, (dti % 4) - 1:(dti % 4) + 1, :])


def build():
    nc = bacc.Bacc("TRN2", target_bir_lowering=False, debug=False,
                   num_devices=N_CORES)
    aps = {}
    for nm in ("xq", "xk", "xv"):
        aps[nm] = nc.dram_tensor(nm, [XR, D], dt.bfloat16, kind="ExternalInput").ap()
    for nm in ("xbq", "xbk", "xbv"):
        aps[nm] = nc.dram_tensor(nm, [32, D], dt.bfloat16, kind="ExternalInput").ap()
    for nm in ("wq", "wk", "wv"):
        aps[nm] = nc.dram_tensor(nm, [D, C], dt.bfloat16, kind="ExternalInput").ap()
    aps["wc"] = nc.dram_tensor("wc", [C, D], dt.bfloat16, kind="ExternalInput").ap()
    aps["wup"] = nc.dram_tensor("wup", [DD, DD], dt.bfloat16, kind="ExternalInput").ap()
    aps["mask"] = nc.dram_tensor("mask", [P, P], dt.bfloat16, kind="ExternalInput").ap()
    aps["taps"] = nc.dram_tensor("taps", [P, NCT * 3 * 4], dt.float32,
                                 kind="ExternalInput").ap()
    aps["bup2"] = nc.dram_tensor("bup2", [P, 1], dt.float32, kind="ExternalInput").ap()
    aps["pm"] = nc.dram_tensor("pm", [P, 48], dt.bfloat16, kind="ExternalInput").ap()
    aps["pms"] = nc.dram_tensor("pms", [32, 32], dt.bfloat16, kind="ExternalInput").ap()
    aps["yT"] = nc.dram_tensor("yT", [D, NP], dt.bfloat16, kind="ExternalOutput").ap()
    with tile.TileContext(nc) as tc:
        _emit(nc, tc, aps)
    nc.compile()
    return nc


_BUILT = None


def _get_built():
    global _BUILT
    if _BUILT is None:
        _BUILT = build()
    return _BUILT


def make_in_maps(q, k, v, Wq, bq, Wk, bk, Wv, bv, Wup, bup, Wc, bc,
                 wcq, bcq, wck, bck, wcv, bcv):
    bf = ml_dtypes.bfloat16
    q, k, v = (np.asarray(x, np.float32) for x in (q, k, v))
    mask_np = (-30.0 * np.tril(np.ones((P, P), np.float32), -1)).astype(bf)
    pm_np, pms_np = _pool_mats()

    def xpad(x):  # [S, D] -> [XR, D] bf16 (9 zero rows + x[0:XR-9])
        out = np.zeros((XR, D), np.float32)
        out[9:XR] = x[0:XR - 9]
        return out.astype(bf)

    def xbound(x):  # [32, D]: row p*16+(s-1) = xpad[128*s + p] = x[128*s+p-9]
        out = np.zeros((32, D), np.float32)
        for p in range(2):
            for si in range(16):
                out[p * 16 + si] = x[128 * (si + 1) + p - 9]
        return out.astype(bf)

    in_maps = []
    for core in range(N_CORES):
        b, half = core // 2, core % 2
        cs = slice(half * C, half * C + C)
        taps = np.zeros((P, NCT, 3, 4), np.float32)
        for oct in range(NCT):
            ch = slice(half * C + oct * P, half * C + (oct + 1) * P)
            for pi, (cw, cb, db, sc) in enumerate((
                    (wcq, bcq, bq, NORM), (wck, bck, bk, NORM), (wcv, bcv, bv, 1.0))):
                w0, w1, w2 = np.asarray(cw, np.float32)[:, ch]
                taps[:, oct, pi, 0] = w0 * sc / KP
                taps[:, oct, pi, 1] = w1 * sc / KP
                taps[:, oct, pi, 2] = w2 * sc / KP
                taps[:, oct, pi, 3] = ((w0 + w1 + w2) * np.asarray(db, np.float32)[ch]
                                       * sc + np.asarray(cb, np.float32)[ch])
        in_maps.append({
            "xq": xpad(q[b]),
            "xk": xpad(k[b]),
            "xv": xpad(v[b]),
            "xbq": xbound(q[b]),
            "xbk": xbound(k[b]),
            "xbv": xbound(v[b]),
            "wq": np.asarray(Wq, np.float32)[:, cs].astype(bf),
            "wk": np.asarray(Wk, np.float32)[:, cs].astype(bf),
            "wv": np.asarray(Wv, np.float32)[:, cs].astype(bf),
            "wc": np.asarray(Wc, np.float32)[cs, :].astype(bf),
            "wup": np.asarray(Wup, np.float32).astype(bf),
            "mask": mask_np,
            "taps": taps.reshape(P, NCT * 3 * 4),
            "bup2": np.tile(np.asarray(bup, np.float32), 2).reshape(P, 1),
            "pm": pm_np,
            "pms": pms_np,
        })
    return in_maps


def gather(results, bc):
    out = np.empty((B, S, D), np.float32)
    bc = np.asarray(bc, np.float32)
    for b in range(B):
        y = (np.asarray(results[2 * b]["yT"], np.float32)
             + np.asarray(results[2 * b + 1]["yT"], np.float32))  # [D, NP]
        out[b] = np.repeat(y.T, KP, axis=0) + bc[None, :]
    return out


def kernel(q, k, v, Wq, bq, Wk, bk, Wv, bv, Wup, bup, Wc, bc,
           wcq, bcq, wck, bck, wcv, bcv):
    nc = _get_built()
    in_maps = make_in_maps(q, k, v, Wq, bq, Wk, bk, Wv, bv, Wup, bup, Wc, bc,
                           wcq, bcq, wck, bck, wcv, bcv)
    res = run_bass_kernel_spmd(nc, in_maps, core_ids=list(range(N_CORES)),
                               trace=False)
    return gather(res.results, bc)
